# revision 7
# baseline (speedup 1.0000x reference)
"""Trainium2 Bass kernel for a per-channel-pair 2x2 unitary mixing layer.

Math (reference): for each channel pair g (C=2048 -> G=1024 pairs):
    M[g] = R(phase[g]) @ polar_project(W_pairs[g])        # 2x2
    y[..., 2g:2g+2] = M[g] @ x[..., 2g:2g+2]

Device formulation (token-major, pure elementwise):
    A[2g] = M[g,0,0]; A[2g+1] = M[g,1,1]
    B[2g] = M[g,0,1]; B[2g+1] = M[g,1,0]
    y[c] = A[c] * x[c] + B[c] * x[partner(c)]
The tiny 2x2 SVD/compose prep is done on host (same formula as the
reference); the [B,T,C] transform runs on 8 NeuronCores, data parallel
over the batch dim (x[b] per core), with the A/B coefficient vectors
replicated (pre-broadcast to [128, C]).

Per core: DVE does the two per-channel multiplies (tensor_tensor, never
contends with GPSIMD's SBUF port), GPSIMD does the add, all DMAs are
HWDGE (nc.sync) in 4 MiB slabs.
"""

import sys

if "/opt/trn_rl_repo" not in sys.path:
    sys.path.insert(0, "/opt/trn_rl_repo")

import numpy as np

# Problem shape (hardcoded per harness contract)
B_FULL, T_FULL, C = 8, 4096, 2048
G = C // 2
N_CORES = 8
TOK = T_FULL  # tokens per core after batch sharding: 4096
P = 128  # SBUF partitions
S = 4  # token-stripes per slab
SS = TOK // (S * P)  # slabs per core: 8

_NC_CACHE = {}


def _prep_coeffs(W_pairs: np.ndarray, phase: np.ndarray):
    """Host prep: M = R(phase) @ polar(W); returns A, B per-channel vectors."""
    W = np.asarray(W_pairs, dtype=np.float32)
    ph = np.asarray(phase, dtype=np.float32)
    # Same math as the reference: polar factor via SVD (U @ Vh), fp32.
    U, _, Vh = np.linalg.svd(W)
    Q = U @ Vh  # [G,2,2]
    c, s = np.cos(ph), np.sin(ph)
    R = np.stack([np.stack([c, -s], -1), np.stack([s, c], -1)], -2)  # [G,2,2]
    M = np.einsum("gij,gjk->gik", R, Q).astype(np.float32)  # [G,2,2]
    A = np.empty(C, dtype=np.float32)
    Bc = np.empty(C, dtype=np.float32)
    A[0::2] = M[:, 0, 0]
    A[1::2] = M[:, 1, 1]
    Bc[0::2] = M[:, 0, 1]
    Bc[1::2] = M[:, 1, 0]
    return A, Bc


def _build_nc():
    """Build the single-core Bass program (SPMD across 8 cores)."""
    if "nc" in _NC_CACHE:
        return _NC_CACHE["nc"]

    import concourse.bacc as bacc
    import concourse.mybir as mybir
    from concourse.tile import TileContext

    f32 = mybir.dt.float32
    mult = mybir.AluOpType.mult
    add = mybir.AluOpType.add

    nc = bacc.Bacc(None)
    x = nc.declare_dram_parameter("x", [TOK, C], f32, isOutput=False)
    ca = nc.declare_dram_parameter("coef_a", [P, C], f32, isOutput=False)
    cb = nc.declare_dram_parameter("coef_b", [P, C], f32, isOutput=False)
    y = nc.declare_dram_parameter("y", [TOK, C], f32, isOutput=True)

    # [TOK, C] viewed as [SS, P, S*C]: slab ss, partition p = token row
    # (ss*S*P + s*P + p), free = (s, c). Each slab is a contiguous 4 MiB
    # DRAM region; each partition's chunks are 8 KiB contiguous.
    xv = x[:, :].rearrange("(ss s p) c -> ss p s c", p=P, s=S)
    yv = y[:, :].rearrange("(ss s p) c -> ss p s c", p=P, s=S)

    with TileContext(nc) as tc:
        with (
            tc.tile_pool(name="coef", bufs=1) as coefp,
            tc.tile_pool(name="xp", bufs=2) as xp,
            tc.tile_pool(name="yp", bufs=2) as yp,
            tc.tile_pool(name="tp", bufs=3) as tp,
            tc.tile_pool(name="up", bufs=3) as up,
        ):
            a_sb = coefp.tile([P, C], f32)
            b_sb = coefp.tile([P, C], f32)
            nc.sync.dma_start(a_sb[:], ca[:, :])
            nc.sync.dma_start(b_sb[:], cb[:, :])
            b3 = b_sb[:].rearrange("p (g two) -> p g two", two=2)
            for ss in range(SS):
                x_sb = xp.tile([P, S * C], f32)
                nc.sync.dma_start(
                    x_sb[:].rearrange("p (s c) -> p s c", s=S), xv[ss]
                )
                y_sb = yp.tile([P, S * C], f32)
                for s in range(S):
                    xs = x_sb[:, s * C : (s + 1) * C]
                    # pair-swap view of the stripe: [...,2g]<->[...,2g+1]
                    xs_sw = xs.rearrange("p (g two) -> p g two", two=2)[:, :, ::-1]
                    t = tp.tile([P, C], f32)
                    u = up.tile([P, C], f32)
                    nc.vector.tensor_tensor(t[:], xs, a_sb[:], mult)
                    nc.vector.tensor_tensor(
                        u[:].rearrange("p (g two) -> p g two", two=2), xs_sw, b3, mult
                    )
                    nc.gpsimd.tensor_tensor(
                        y_sb[:, s * C : (s + 1) * C], t[:], u[:], add
                    )
                nc.sync.dma_start(
                    yv[ss], y_sb[:].rearrange("p (s c) -> p s c", s=S)
                )

    nc.finalize()
    _NC_CACHE["nc"] = nc
    return nc


def run(x, W_pairs, phase, trace=False):
    """Run on 8 NeuronCores; returns (y_full, BassKernelResults)."""
    from concourse.bass_utils import run_bass_kernel_spmd

    x = np.ascontiguousarray(np.asarray(x, dtype=np.float32))
    assert x.shape == (B_FULL, T_FULL, C), x.shape
    A, Bc = _prep_coeffs(W_pairs, phase)
    a_bc = np.ascontiguousarray(np.broadcast_to(A, (P, C)))
    b_bc = np.ascontiguousarray(np.broadcast_to(Bc, (P, C)))

    nc = _build_nc()
    in_maps = [
        {"x": x[core].reshape(TOK, C), "coef_a": a_bc, "coef_b": b_bc}
        for core in range(N_CORES)
    ]
    res = run_bass_kernel_spmd(nc, in_maps, list(range(N_CORES)), trace=trace)
    y = np.stack([res.results[i]["y"] for i in range(N_CORES)], axis=0)
    return y.reshape(B_FULL, T_FULL, C), res


def kernel(x, W_pairs, phase):
    y, _ = run(x, W_pairs, phase)
    return y


# revision 8
# speedup vs baseline: 1.4277x; 1.4277x over previous
"""Trainium2 Bass kernel for a per-channel-pair 2x2 unitary mixing layer.

Math (reference): for each channel pair g (C=2048 -> G=1024 pairs):
    M[g] = R(phase[g]) @ polar_project(W_pairs[g])        # 2x2
    y[..., 2g:2g+2] = M[g] @ x[..., 2g:2g+2]

Device formulation (token-major, pure elementwise):
    A[2g] = M[g,0,0]; A[2g+1] = M[g,1,1]
    B[2g] = M[g,0,1]; B[2g+1] = M[g,1,0]
    y[c] = A[c] * x[c] + B[c] * x[partner(c)]
The tiny 2x2 SVD/compose prep is done on host (same formula as the
reference); the [B,T,C] transform runs on 8 NeuronCores, data parallel
over the batch dim (x[b] per core), with the A/B coefficient vectors
replicated (pre-broadcast to [128, C]).

Per core: DVE does the two per-channel multiplies (tensor_tensor, never
contends with GPSIMD's SBUF port), GPSIMD does the add, all DMAs are
HWDGE (nc.sync) in 4 MiB slabs.
"""

import sys

if "/opt/trn_rl_repo" not in sys.path:
    sys.path.insert(0, "/opt/trn_rl_repo")

import numpy as np

# Problem shape (hardcoded per harness contract)
B_FULL, T_FULL, C = 8, 4096, 2048
G = C // 2
N_CORES = 8
TOK = T_FULL  # tokens per core after batch sharding: 4096
P = 128  # SBUF partitions
S = 4  # token-stripes per slab
SS = TOK // (S * P)  # slabs per core: 8

_NC_CACHE = {}


def _prep_coeffs(W_pairs: np.ndarray, phase: np.ndarray):
    """Host prep: M = R(phase) @ polar(W); returns A, B per-channel vectors."""
    W = np.asarray(W_pairs, dtype=np.float32)
    ph = np.asarray(phase, dtype=np.float32)
    # Same math as the reference: polar factor via SVD (U @ Vh), fp32.
    U, _, Vh = np.linalg.svd(W)
    Q = U @ Vh  # [G,2,2]
    c, s = np.cos(ph), np.sin(ph)
    R = np.stack([np.stack([c, -s], -1), np.stack([s, c], -1)], -2)  # [G,2,2]
    M = np.einsum("gij,gjk->gik", R, Q).astype(np.float32)  # [G,2,2]
    A = np.empty(C, dtype=np.float32)
    Bc = np.empty(C, dtype=np.float32)
    A[0::2] = M[:, 0, 0]
    A[1::2] = M[:, 1, 1]
    Bc[0::2] = M[:, 0, 1]
    Bc[1::2] = M[:, 1, 0]
    return A, Bc


def _build_nc():
    """Build the single-core Bass program (SPMD across 8 cores)."""
    if "nc" in _NC_CACHE:
        return _NC_CACHE["nc"]

    import concourse.bacc as bacc
    import concourse.mybir as mybir
    from concourse.tile import TileContext

    f32 = mybir.dt.float32
    mult = mybir.AluOpType.mult
    add = mybir.AluOpType.add

    nc = bacc.Bacc(None)
    x = nc.declare_dram_parameter("x", [TOK, C], f32, isOutput=False)
    ca = nc.declare_dram_parameter("coef_a", [P, C], f32, isOutput=False)
    cb = nc.declare_dram_parameter("coef_b", [P, C], f32, isOutput=False)
    y = nc.declare_dram_parameter("y", [TOK, C], f32, isOutput=True)

    # [TOK, C] viewed as [SS, P, S*C]: slab ss, partition p = token row
    # (ss*S*P + s*P + p), free = (s, c). Each slab is a contiguous 4 MiB
    # DRAM region; each partition's chunks are 8 KiB contiguous.
    xv = x[:, :].rearrange("(ss s p) c -> ss p s c", p=P, s=S)
    yv = y[:, :].rearrange("(ss s p) c -> ss p s c", p=P, s=S)

    with TileContext(nc) as tc:
        with (
            tc.tile_pool(name="coef", bufs=1) as coefp,
            tc.tile_pool(name="xp", bufs=2) as xp,
            tc.tile_pool(name="yp", bufs=2) as yp,
            tc.tile_pool(name="tp", bufs=3) as tp,
            tc.tile_pool(name="up", bufs=3) as up,
        ):
            a_sb = coefp.tile([P, C], f32)
            b_sb = coefp.tile([P, C], f32)
            nc.sync.dma_start(a_sb[:], ca[:, :])
            nc.sync.dma_start(b_sb[:], cb[:, :])
            b3 = b_sb[:].rearrange("p (g two) -> p g two", two=2)
            for ss in range(SS):
                x_sb = xp.tile([P, S * C], f32)
                nc.sync.dma_start(
                    x_sb[:].rearrange("p (s c) -> p s c", s=S), xv[ss]
                )
                y_sb = yp.tile([P, S * C], f32)
                for s in range(S):
                    xs = x_sb[:, s * C : (s + 1) * C]
                    # pair-swap view of the stripe: [...,2g]<->[...,2g+1]
                    xs_sw = xs.rearrange("p (g two) -> p g two", two=2)[:, :, ::-1]
                    t = tp.tile([P, C], f32)
                    u = up.tile([P, C], f32)
                    nc.vector.tensor_tensor(t[:], xs, a_sb[:], mult)
                    nc.vector.tensor_tensor(
                        u[:].rearrange("p (g two) -> p g two", two=2), xs_sw, b3, mult
                    )
                    nc.vector.tensor_tensor(
                        y_sb[:, s * C : (s + 1) * C], t[:], u[:], add
                    )
                nc.sync.dma_start(
                    yv[ss], y_sb[:].rearrange("p (s c) -> p s c", s=S)
                )

    nc.finalize()
    _NC_CACHE["nc"] = nc
    return nc


def run(x, W_pairs, phase, trace=False):
    """Run on 8 NeuronCores; returns (y_full, BassKernelResults)."""
    from concourse.bass_utils import run_bass_kernel_spmd

    x = np.ascontiguousarray(np.asarray(x, dtype=np.float32))
    assert x.shape == (B_FULL, T_FULL, C), x.shape
    A, Bc = _prep_coeffs(W_pairs, phase)
    a_bc = np.ascontiguousarray(np.broadcast_to(A, (P, C)))
    b_bc = np.ascontiguousarray(np.broadcast_to(Bc, (P, C)))

    nc = _build_nc()
    in_maps = [
        {"x": x[core].reshape(TOK, C), "coef_a": a_bc, "coef_b": b_bc}
        for core in range(N_CORES)
    ]
    res = run_bass_kernel_spmd(nc, in_maps, list(range(N_CORES)), trace=trace)
    y = np.stack([res.results[i]["y"] for i in range(N_CORES)], axis=0)
    return y.reshape(B_FULL, T_FULL, C), res


def kernel(x, W_pairs, phase):
    y, _ = run(x, W_pairs, phase)
    return y


# revision 9
# speedup vs baseline: 1.5842x; 1.1096x over previous
"""Trainium2 Bass kernel for a per-channel-pair 2x2 unitary mixing layer.

Math (reference): for each channel pair g (C=2048 -> G=1024 pairs):
    M[g] = R(phase[g]) @ polar_project(W_pairs[g])        # 2x2
    y[..., 2g:2g+2] = M[g] @ x[..., 2g:2g+2]

Device formulation (token-major):
    A[2g] = M[g,0,0]; A[2g+1] = M[g,1,1]
    B[2g] = M[g,0,1]; B[2g+1] = M[g,1,0]
    y[c] = A[c] * x[c] + B[c] * x[partner(c)]
The tiny 2x2 SVD/compose prep runs on host (same formula as the
reference); the [B,T,C] transform runs on 8 NeuronCores, data parallel
over batch (x[b] per core), with coefficient tensors replicated.

Per core the work is split across engine pipelines so the kernel rides
the HBM roofline (~206 us for 64 MiB at the per-NC limit):
  - DVE path (6 of 8 slabs): three fp32 tensor_tensor ops per stripe
    (t = x*A; u = swap(x)*B; y = t + u). ~27.5 us/slab.
  - PE path (2 of 8 slabs): per 128x128 block, PE transpose to PSUM,
    ACT copy to SBUF, PE matmul against the host-built block-diagonal
    W chunk (y = x @ Wblk), ACT copy back. ~45 us/slab, runs on
    otherwise-idle engines.
GPSIMD is left idle: its SBUF port serializes against DVE 2-read ops
(exclusive shared-port lock, measured). All DMAs are HWDGE (nc.sync),
4 MiB per transfer.
"""

import sys

if "/opt/trn_rl_repo" not in sys.path:
    sys.path.insert(0, "/opt/trn_rl_repo")

import numpy as np

# Problem shape (hardcoded per harness contract)
B_FULL, T_FULL, C = 8, 4096, 2048
G = C // 2
N_CORES = 8
TOK = T_FULL  # tokens per core after batch sharding: 4096
P = 128  # SBUF partitions
S = 4  # token-stripes per slab
SS = TOK // (S * P)  # slabs per core: 8
NBLK = C // P  # 128-channel blocks per stripe: 16

# Slabs processed on the PE (transpose+matmul) path; rest on DVE path.
PE_SLABS = (2, 5)

_NC_CACHE = {}


def _prep_coeffs(W_pairs: np.ndarray, phase: np.ndarray):
    """Host prep: M = R(phase) @ polar(W).

    Returns per-channel coefficient vectors A, B and the block-diagonal
    weight chunks W_dram [P, C] used by the PE path (W_dram[p, j*128+n]
    = Wblk_j[p, n], y_block = x_block @ Wblk_j).
    """
    W = np.asarray(W_pairs, dtype=np.float32)
    ph = np.asarray(phase, dtype=np.float32)
    # Same math as the reference: polar factor via SVD (U @ Vh), fp32.
    U, _, Vh = np.linalg.svd(W)
    Q = U @ Vh  # [G,2,2]
    c, s = np.cos(ph), np.sin(ph)
    R = np.stack([np.stack([c, -s], -1), np.stack([s, c], -1)], -2)  # [G,2,2]
    M = np.einsum("gij,gjk->gik", R, Q).astype(np.float32)  # [G,2,2]

    A = np.empty(C, dtype=np.float32)
    Bc = np.empty(C, dtype=np.float32)
    A[0::2] = M[:, 0, 0]
    A[1::2] = M[:, 1, 1]
    Bc[0::2] = M[:, 0, 1]
    Bc[1::2] = M[:, 1, 0]

    # Block-diagonal chunks: Wblk_j[2gl+jj, 2gl+ii] = M[j*64+gl, ii, jj]
    Wblk = np.zeros((NBLK, P, P), dtype=np.float32)
    gl = np.arange(P // 2)
    for j in range(NBLK):
        Mj = M[j * (P // 2) + gl]  # [64,2,2]
        for jj in range(2):
            for ii in range(2):
                Wblk[j, 2 * gl + jj, 2 * gl + ii] = Mj[:, ii, jj]
    W_dram = np.ascontiguousarray(Wblk.transpose(1, 0, 2).reshape(P, C))
    return A, Bc, W_dram


def _build_nc():
    """Build the single-core Bass program (SPMD across 8 cores)."""
    if "nc" in _NC_CACHE:
        return _NC_CACHE["nc"]

    import concourse.bacc as bacc
    import concourse.mybir as mybir
    from concourse.tile import TileContext

    f32 = mybir.dt.float32
    mult = mybir.AluOpType.mult
    add = mybir.AluOpType.add

    nc = bacc.Bacc(None)
    x = nc.declare_dram_parameter("x", [TOK, C], f32, isOutput=False)
    ca = nc.declare_dram_parameter("coef_a", [P, C], f32, isOutput=False)
    cb = nc.declare_dram_parameter("coef_b", [P, C], f32, isOutput=False)
    w = nc.declare_dram_parameter("wblk", [P, C], f32, isOutput=False)
    ident = nc.declare_dram_parameter("ident", [P, P], f32, isOutput=False)
    y = nc.declare_dram_parameter("y", [TOK, C], f32, isOutput=True)

    # [TOK, C] viewed as [SS, P, S, C]: slab ss, partition p = token row
    # (ss*S*P + s*P + p). Each slab is a contiguous 4 MiB DRAM region;
    # per-partition chunks are 8 KiB contiguous.
    xv = x[:, :].rearrange("(ss s p) c -> ss p s c", p=P, s=S)
    yv = y[:, :].rearrange("(ss s p) c -> ss p s c", p=P, s=S)

    with TileContext(nc) as tc:
        with (
            tc.tile_pool(name="coef", bufs=1) as coefp,
            tc.tile_pool(name="xp", bufs=2) as xp,
            tc.tile_pool(name="yp", bufs=2) as yp,
            tc.tile_pool(name="tp", bufs=2) as tp,
            tc.tile_pool(name="up", bufs=2) as up,
            tc.tile_pool(name="xtp", bufs=3) as xtp,
            tc.tile_pool(name="pst", bufs=2, space="PSUM") as pst,
            tc.tile_pool(name="psy", bufs=2, space="PSUM") as psy,
        ):
            a_sb = coefp.tile([P, C], f32)
            b_sb = coefp.tile([P, C], f32)
            w_sb = coefp.tile([P, C], f32)
            id_sb = coefp.tile([P, P], f32)
            nc.sync.dma_start(a_sb[:], ca[:, :])
            nc.sync.dma_start(b_sb[:], cb[:, :])
            nc.sync.dma_start(w_sb[:], w[:, :])
            nc.sync.dma_start(id_sb[:], ident[:, :])
            b3 = b_sb[:].rearrange("p (g two) -> p g two", two=2)

            for ss in range(SS):
                x_sb = xp.tile([P, S * C], f32)
                nc.sync.dma_start(
                    x_sb[:].rearrange("p (s c) -> p s c", s=S), xv[ss]
                )
                y_sb = yp.tile([P, S * C], f32)
                if ss in PE_SLABS:
                    # PE path: per 128x128 block, transpose then matmul.
                    for s in range(S):
                        for a in range(NBLK // 4):
                            psT = pst.tile([P, 512], f32)
                            for b in range(4):
                                j = 4 * a + b
                                xblk = x_sb[
                                    :, s * C + j * P : s * C + (j + 1) * P
                                ]
                                nc.tensor.transpose(
                                    psT[:, b * P : (b + 1) * P], xblk, id_sb[:]
                                )
                            xt = xtp.tile([P, 512], f32)
                            nc.scalar.copy(xt[:], psT[:])
                            psY = psy.tile([P, 512], f32)
                            for b in range(4):
                                j = 4 * a + b
                                nc.tensor.matmul(
                                    psY[:, b * P : (b + 1) * P],
                                    xt[:, b * P : (b + 1) * P],
                                    w_sb[:, j * P : (j + 1) * P],
                                )
                            nc.scalar.copy(
                                y_sb[:, s * C + a * 512 : s * C + (a + 1) * 512],
                                psY[:],
                            )
                else:
                    # DVE path: three fp32 TT ops per stripe.
                    for s in range(S):
                        xs = x_sb[:, s * C : (s + 1) * C]
                        xs_sw = xs.rearrange("p (g two) -> p g two", two=2)[
                            :, :, ::-1
                        ]
                        t = tp.tile([P, C], f32)
                        u = up.tile([P, C], f32)
                        nc.vector.tensor_tensor(t[:], xs, a_sb[:], mult)
                        nc.vector.tensor_tensor(
                            u[:].rearrange("p (g two) -> p g two", two=2),
                            xs_sw,
                            b3,
                            mult,
                        )
                        nc.vector.tensor_tensor(
                            y_sb[:, s * C : (s + 1) * C], t[:], u[:], add
                        )
                nc.sync.dma_start(
                    yv[ss], y_sb[:].rearrange("p (s c) -> p s c", s=S)
                )

    nc.finalize()
    _NC_CACHE["nc"] = nc
    return nc


def run(x, W_pairs, phase, trace=False):
    """Run on 8 NeuronCores; returns (y_full, BassKernelResults)."""
    from concourse.bass_utils import run_bass_kernel_spmd

    x = np.ascontiguousarray(np.asarray(x, dtype=np.float32))
    assert x.shape == (B_FULL, T_FULL, C), x.shape
    A, Bc, W_dram = _prep_coeffs(W_pairs, phase)
    a_bc = np.ascontiguousarray(np.broadcast_to(A, (P, C)))
    b_bc = np.ascontiguousarray(np.broadcast_to(Bc, (P, C)))
    ident = np.eye(P, dtype=np.float32)

    nc = _build_nc()
    in_maps = [
        {
            "x": x[core].reshape(TOK, C),
            "coef_a": a_bc,
            "coef_b": b_bc,
            "wblk": W_dram,
            "ident": ident,
        }
        for core in range(N_CORES)
    ]
    res = run_bass_kernel_spmd(nc, in_maps, list(range(N_CORES)), trace=trace)
    y = np.stack([res.results[i]["y"] for i in range(N_CORES)], axis=0)
    return y.reshape(B_FULL, T_FULL, C), res


def kernel(x, W_pairs, phase):
    y, _ = run(x, W_pairs, phase)
    return y
